# revision 9
# baseline (speedup 1.0000x reference)
"""Multi-head attention forward on 8 TRN2 NeuronCores.

Problem: B=2, L=2048, D=1024, H=16, Hd=64 MHA block:
    qkv = x @ w_qkv + b_qkv ; per-head softmax(q k^T / sqrt(Hd)) @ v ; o @ w_out + b_out

Sharding (tensor parallel over heads x batch):
  core c -> batch c//4, heads [4*(c%4), 4*(c%4)+4).
  Each core computes its 4 heads' attention for its batch and a partial
  out-projection (2048, 1024). Host sums the 4 partials per batch + b_out.

Host-side layout prep: x is pre-transposed per batch (D on partitions),
w_qkv column-sliced per core (q weights pre-scaled by 1/sqrt(Hd)), w_out
row-sliced. Device keeps q^T/k^T with head-dim on partitions so attention
needs no on-device transposes; v is produced token-major and augmented with
a ones column so the PV matmul also accumulates the softmax denominators.
All matmul operands are bitcast to float32r (fp32 memory format, 1 cycle/row
on the PE at free-dim >= 256 vs 4 for plain fp32).
"""

from contextlib import ExitStack

import numpy as np

B, L, D = 2, 2048, 1024
H, HD = 16, 64
NCORES = 8
CORES_PER_BATCH = 4
H_C = H // CORES_PER_BATCH          # heads per core = 4
COLS = H_C * HD                     # qkv cols per core = 256
P = 128
NKT = D // P                        # 8 contraction tiles over D
NQS = L // 512                      # 4 query slices of 512
NKB = L // P                        # 16 key/token blocks of 128
NMB = COLS // P                     # 2 col-blocks of the per-core qkv slice
NDT = COLS // P                     # 2 contraction tiles over per-core o dims
SCALE = 1.0 / np.sqrt(np.float32(HD))

_NC_CACHE = None
LAST_RESULTS = None


def _build_nc():
    import concourse.tile as tile
    from concourse import bacc, mybir

    f32 = mybir.dt.float32
    f32r = mybir.dt.float32r
    Exp = mybir.ActivationFunctionType.Exp

    nc = bacc.Bacc(None, target_bir_lowering=False)

    xt_d = nc.declare_dram_parameter("xt", [NKT, P, L], f32r, isOutput=False)
    wq_d = nc.declare_dram_parameter("wq", [NKT, P, COLS], f32r, isOutput=False)
    wk_d = nc.declare_dram_parameter("wk", [NKT, P, COLS], f32r, isOutput=False)
    wv_d = nc.declare_dram_parameter("wv", [NKT, P, COLS], f32r, isOutput=False)
    bq_d = nc.declare_dram_parameter("bq", [NMB, P, 1], f32, isOutput=False)
    bk_d = nc.declare_dram_parameter("bk", [NMB, P, 1], f32, isOutput=False)
    bv_d = nc.declare_dram_parameter("bv", [1, COLS], f32r, isOutput=False)
    wo_d = nc.declare_dram_parameter("wo", [NDT, P, D], f32r, isOutput=False)
    y_d = nc.declare_dram_parameter("y", [L, D], f32, isOutput=True)

    with tile.TileContext(nc) as tc, ExitStack() as ctx, nc.allow_low_precision("tf32 matmul operands; accumulation stays fp32 in PSUM"):
        consts = ctx.enter_context(tc.tile_pool(name="consts", bufs=1))
        xtp = ctx.enter_context(tc.tile_pool(name="xtp", bufs=NKT))
        wp = ctx.enter_context(tc.tile_pool(name="wp", bufs=NKT))
        bigs = ctx.enter_context(tc.tile_pool(name="bigs", bufs=1))
        pp = ctx.enter_context(tc.tile_pool(name="pp", bufs=4))
        yp = ctx.enter_context(tc.tile_pool(name="yp", bufs=3))
        smallp = ctx.enter_context(tc.tile_pool(name="smallp", bufs=4))
        psum = ctx.enter_context(tc.tile_pool(name="psum", bufs=4, space="PSUM"))
        psum_o = ctx.enter_context(tc.tile_pool(name="psum_o", bufs=2, space="PSUM"))

        # ---- constants ----
        ones_f32 = consts.tile([1, P], f32, tag="ones_f32")
        nc.vector.memset(ones_f32[:], 1.0)
        ones_sb = consts.tile([1, P], f32r, tag="ones")
        nc.vector.tensor_copy(ones_sb[:], ones_f32[:])
        bq_sb = consts.tile([P, NMB], f32, tag="bq")
        bk_sb = consts.tile([P, NMB], f32, tag="bk")
        for mb in range(NMB):
            nc.sync.dma_start(out=bq_sb[:, mb : mb + 1], in_=bq_d[mb])
            nc.sync.dma_start(out=bk_sb[:, mb : mb + 1], in_=bk_d[mb])
        bv_sb = consts.tile([1, COLS], f32r, tag="bv")
        nc.sync.dma_start(out=bv_sb[:], in_=bv_d[:])

        # ---- stream in x^T and weights as per-k-tile tiles ----
        xt_t = [xtp.tile([P, L], f32r, tag="xt", name=f"xt{i}") for i in range(NKT)]
        wq_t = [wp.tile([P, COLS], f32r, tag="wq", name=f"wq{i}") for i in range(NKT)]
        wk_t = [wp.tile([P, COLS], f32r, tag="wk", name=f"wk{i}") for i in range(NKT)]
        wv_t = [wp.tile([P, COLS], f32r, tag="wv", name=f"wv{i}") for i in range(NKT)]
        for kt in range(NKT):
            nc.sync.dma_start(out=wq_t[kt][:], in_=wq_d[kt])
            nc.sync.dma_start(out=wk_t[kt][:], in_=wk_d[kt])
            nc.sync.dma_start(out=wv_t[kt][:], in_=wv_d[kt])
            nc.sync.dma_start(out=xt_t[kt][:], in_=xt_d[kt])
        wo_t = [wp.tile([P, D], f32r, tag="wo", name=f"wo{i}", bufs=NDT) for i in range(NDT)]
        for dt_i in range(NDT):
            nc.sync.dma_start(out=wo_t[dt_i][:], in_=wo_d[dt_i])

        # ---- persistent intermediates ----
        # q^T/k^T: partition = qkv col within a 128-block, dims (col_block, token)
        qt_sb = bigs.tile([P, NMB, L], f32r, tag="qt")
        kt_sb = bigs.tile([P, NMB, L], f32r, tag="kt")
        # v natural + ones column: partition = token within block, (kblock, head, hd+1)
        vx_sb = bigs.tile([P, NKB, H_C, HD + 1], f32r, tag="vx")
        vxones_f32 = consts.tile([P, NKB, H_C, 1], f32, tag="vxones")
        nc.vector.memset(vxones_f32[:], 1.0)
        nc.vector.tensor_copy(vx_sb[:, :, :, HD : HD + 1], vxones_f32[:])
        # normalized attention output, transposed: partition = o-dim within a
        # 128-block, dims (dim_block, token)
        ot_sb = bigs.tile([P, NDT, L], f32r, tag="ot")

        # ---- phase 1: qkv projection ----
        for w_t, b_sb, dst in ((wq_t, bq_sb, qt_sb), (wk_t, bk_sb, kt_sb)):
            for mb in range(NMB):
                for ns in range(NQS):
                    ps = psum.tile([P, 512], f32, tag="mm512", name="ps_qk")
                    for kt in range(NKT):
                        nc.tensor.matmul(
                            ps,
                            lhsT=w_t[kt][:, mb * P : (mb + 1) * P],
                            rhs=xt_t[kt][:, ns * 512 : (ns + 1) * 512],
                            start=(kt == 0),
                            stop=(kt == NKT - 1),
                        )
                    nc.vector.tensor_scalar_add(
                        dst[:, mb, ns * 512 : (ns + 1) * 512], ps, b_sb[:, mb : mb + 1]
                    )
        for tb in range(NKB):
            ps = psum.tile([P, 512], f32, tag="mm512", name="ps_v")[:, :COLS]
            for kt in range(NKT):
                nc.tensor.matmul(
                    ps,
                    lhsT=xt_t[kt][:, tb * P : (tb + 1) * P],
                    rhs=wv_t[kt][:],
                    start=(kt == 0),
                    stop=False,
                )
            # rank-1 bias add: ones(128) x b_v(256)
            nc.tensor.matmul(
                ps,
                lhsT=ones_sb[:],
                rhs=bv_sb[:],
                start=False,
                stop=True,
            )
            nc.vector.tensor_copy(
                vx_sb[:, tb, :, 0:HD],
                ps.rearrange("p (h d) -> p h d", h=H_C),
            )

        # ---- phase 2: attention (scores^T -> exp -> PV w/ augmented v) ----
        for h in range(H_C):
            mb, off = divmod(h, 2)
            off *= HD
            for qs in range(NQS):
                po = psum_o.tile([HD + 1, 512], f32, tag="po", name="po")
                for kb in range(NKB):
                    ps = psum.tile([P, 512], f32, tag="mm512", name="ps_s")
                    nc.tensor.matmul(
                        ps,
                        lhsT=kt_sb[off : off + HD, mb, kb * P : (kb + 1) * P],
                        rhs=qt_sb[off : off + HD, mb, qs * 512 : (qs + 1) * 512],
                        start=True,
                        stop=True,
                    )
                    p_sb = pp.tile([P, 512], f32r, tag="p", name="p_sb")
                    nc.scalar.activation(p_sb[:], ps, Exp)
                    nc.tensor.matmul(
                        po,
                        lhsT=vx_sb[:, kb, h, :],
                        rhs=p_sb[:],
                        start=(kb == 0),
                        stop=(kb == NKB - 1),
                    )
                # normalize: columns of po[0:HD] scaled by 1/po[HD]
                rec = smallp.tile([1, 512], f32r, tag="rec", name="rec")
                nc.vector.reciprocal(rec[:], po[HD : HD + 1, :])
                pb = psum.tile([P, 512], f32, tag="mm512", name="pb")[0:HD]
                nc.tensor.matmul(
                    pb,
                    lhsT=ones_sb[:, 0:HD],
                    rhs=rec[:],
                    start=True,
                    stop=True,
                )
                # DVE can read only one PSUM operand: stage the broadcast in SBUF
                pb_sb = smallp.tile([HD, 512], f32, tag="pb_sb", name="pb_sb")
                nc.vector.tensor_copy(pb_sb[:], pb)
                nc.vector.tensor_mul(
                    ot_sb[off : off + HD, mb, qs * 512 : (qs + 1) * 512],
                    po[0:HD, :],
                    pb_sb[:],
                )

        # ---- phase 3: partial out-projection ----
        for tb in range(NKB):
            for nb in range(D // 512):
                ps = psum.tile([P, 512], f32, tag="mm512", name="ps_y")
                for dt_i in range(NDT):
                    nc.tensor.matmul(
                        ps,
                        lhsT=ot_sb[:, dt_i, tb * P : (tb + 1) * P],
                        rhs=wo_t[dt_i][:, nb * 512 : (nb + 1) * 512],
                        start=(dt_i == 0),
                        stop=(dt_i == NDT - 1),
                    )
                y_sb = yp.tile([P, 512], f32, tag="y", name="y_sb")
                nc.vector.tensor_copy(y_sb[:], ps)
                nc.sync.dma_start(
                    out=y_d[tb * P : (tb + 1) * P, nb * 512 : (nb + 1) * 512],
                    in_=y_sb[:],
                )

    nc.finalize()
    return nc


def get_nc():
    global _NC_CACHE
    if _NC_CACHE is None:
        _NC_CACHE = _build_nc()
    return _NC_CACHE


def make_in_maps(x, w_qkv, b_qkv, w_out):
    x = np.ascontiguousarray(np.asarray(x, dtype=np.float32))
    w_qkv = np.ascontiguousarray(np.asarray(w_qkv, dtype=np.float32))
    b_qkv = np.ascontiguousarray(np.asarray(b_qkv, dtype=np.float32))
    w_out = np.ascontiguousarray(np.asarray(w_out, dtype=np.float32))

    in_maps = []
    for c in range(NCORES):
        b, g = divmod(c, CORES_PER_BATCH)
        cs, ce = g * COLS, (g + 1) * COLS
        xt = np.ascontiguousarray(x[b].T).reshape(NKT, P, L)
        wq = np.ascontiguousarray(w_qkv[:, 0 * D : 1 * D][:, cs:ce] * SCALE).reshape(
            NKT, P, COLS
        )
        wk = np.ascontiguousarray(w_qkv[:, 1 * D : 2 * D][:, cs:ce]).reshape(
            NKT, P, COLS
        )
        wv = np.ascontiguousarray(w_qkv[:, 2 * D : 3 * D][:, cs:ce]).reshape(
            NKT, P, COLS
        )
        bq = np.ascontiguousarray(b_qkv[0 * D : 1 * D][cs:ce] * SCALE).reshape(
            NMB, P, 1
        )
        bk = np.ascontiguousarray(b_qkv[1 * D : 2 * D][cs:ce]).reshape(NMB, P, 1)
        bv = np.ascontiguousarray(b_qkv[2 * D : 3 * D][cs:ce]).reshape(1, COLS)
        wo = np.ascontiguousarray(w_out[cs:ce, :]).reshape(NDT, P, D)
        in_maps.append(
            dict(xt=xt, wq=wq, wk=wk, wv=wv, bq=bq, bk=bk, bv=bv, wo=wo)
        )
    return in_maps


def kernel(x, w_qkv, b_qkv, w_out, b_out, _trace=False, **_kw):
    global LAST_RESULTS
    from concourse.bass_utils import run_bass_kernel_spmd

    nc = get_nc()
    in_maps = make_in_maps(x, w_qkv, b_qkv, w_out)
    res = run_bass_kernel_spmd(nc, in_maps, list(range(NCORES)), trace=_trace, **_kw)
    LAST_RESULTS = res

    b_out = np.asarray(b_out, dtype=np.float32)
    y = np.zeros((B, L, D), dtype=np.float32)
    for c in range(NCORES):
        y[c // CORES_PER_BATCH] += res.results[c]["y"]
    y += b_out[None, None, :]
    return y
